# revision 1
# baseline (speedup 1.0000x reference)
"""CircleLayer (histogram angle binning) Trainium2 Bass kernel.

Full-input contract: kernel(**inputs) takes the complete arrays, shards the
batch dim across 8 NeuronCores (pure data parallel), runs one SPMD Bass
program, and gathers the full [B, P, 2*D] output.

Per-core layout (512 samples = 4 tiles of 128):
  - geometry phase in [128 samples(part), 128 neighbors(free)] tiles on DVE/ACT
  - angle bins via exact fp32 thresholds (precomputed to match the reference's
    fp32 divide + int32 trunc semantics bit-exactly)
  - onehot (scaled by 1/n) transposed on PE, then 128 per-sample matmuls
    onehotS[128n, 8].T @ f_res[128n, 64] -> PSUM (16 samples packed per bank)
  - f_scan = relu(scan @ W_ce + b_ce) computed batched on DVE + one ACT relu
"""

import numpy as np

B, N, T, D = 4096, 128, 20, 64
P = 8
NCORES = 8
BC = B // NCORES  # samples per core
TILE = 128
NT = BC // TILE  # tiles per core

PI32 = np.float32(np.pi)
TWOPI32 = np.float32(2.0 * np.pi)
C32 = np.float32((2.0 * np.pi) / P)  # bin width as the reference computes it


def _bin_thresholds():
    """T[p] = smallest fp32 x >= 0 with int32(fp32(x / C32)) >= p.

    Comparing dir >= T[p] then reproduces the reference's
    (dir / C32).astype(int32) binning exactly (fp32 division is monotone).
    """
    thr = [np.float32(0.0)]
    for p in range(1, P + 1):
        x = np.float32(np.float32(p) * C32)
        while int(np.float32(x / C32)) >= p:
            x = np.nextafter(x, np.float32(-np.inf))
        while int(np.float32(x / C32)) < p:
            x = np.nextafter(x, np.float32(np.inf))
        thr.append(np.float32(x))
    return thr


THR = _bin_thresholds()

_prog_cache = {}


def _build_program():
    import concourse.bass as bass
    import concourse.tile as tile
    from concourse import bacc, mybir
    from concourse.masks import make_identity

    f32 = mybir.dt.float32
    AX = mybir.AxisListType
    OP = mybir.AluOpType
    AF = mybir.ActivationFunctionType

    nc = bacc.Bacc(
        "TRN2",
        target_bir_lowering=False,
        debug=False,
        enable_asserts=False,
        num_devices=NCORES,
    )

    nei = nc.dram_tensor("nei", [BC, N * T * 2], f32, kind="ExternalInput").ap()
    fresT = nc.dram_tensor("fresT", [N, BC * D], f32, kind="ExternalInput").ap()
    egoR = nc.dram_tensor("egoR", [TILE, NT * 2], f32, kind="ExternalInput").ap()
    wb = nc.dram_tensor("wb", [TILE, 3 * D], f32, kind="ExternalInput").ap()
    res_out = nc.dram_tensor("res_out", [128, NT * 512], f32, kind="ExternalOutput").ap()
    fscan_out = nc.dram_tensor("fscan_out", [BC, P * D], f32, kind="ExternalOutput").ap()

    FREE_NEI = N * T * 2  # 10240 floats per sample

    with tile.TileContext(nc) as tc:
        with (
            tc.tile_pool(name="const", bufs=1) as constp,
            tc.tile_pool(name="nei", bufs=2) as neip,
            tc.tile_pool(name="fres", bufs=2) as fresp,
            tc.tile_pool(name="geo", bufs=2) as geo,
            tc.tile_pool(name="small", bufs=2) as small,
            tc.tile_pool(name="oht", bufs=2) as ohtp,
            tc.tile_pool(name="tpsum", bufs=2, space="PSUM") as tpsum,
            tc.tile_pool(name="opsum", bufs=4, space="PSUM") as opsum,
        ):
            ident = constp.tile([128, 128], f32)
            make_identity(nc, ident[:])
            ego_sb = constp.tile([TILE, NT * 2], f32)
            nc.sync.dma_start(out=ego_sb[:], in_=egoR)
            wb_sb = constp.tile([TILE, 3 * D], f32)
            nc.sync.dma_start(out=wb_sb[:], in_=wb)
            w0 = wb_sb[:, 0:D]
            w1 = wb_sb[:, D : 2 * D]
            bias = wb_sb[:, 2 * D : 3 * D]

            for t in range(NT):
                rows = slice(t * TILE, (t + 1) * TILE)

                nei_sb = neip.tile([TILE, FREE_NEI], f32)
                nc.sync.dma_start(out=nei_sb[:], in_=nei[rows, :])
                nei_v = nei_sb[:].rearrange("b (n f) -> b n f", f=T * 2)

                fres_sb = fresp.tile([N, TILE * D], f32)
                nc.sync.dma_start(
                    out=fres_sb[:],
                    in_=fresT[:, t * TILE * D : (t + 1) * TILE * D],
                )

                # --- geometry ---
                msum = geo.tile([TILE, N], f32)
                nc.vector.tensor_reduce(msum[:], nei_v, axis=AX.X, op=OP.add)

                egox = ego_sb[:, 2 * t : 2 * t + 1]
                egoy = ego_sb[:, 2 * t + 1 : 2 * t + 2]
                relx = geo.tile([TILE, N], f32)
                nc.vector.tensor_scalar(relx[:], nei_v[:, :, 2 * T - 2], egox, None, OP.subtract)
                rely = geo.tile([TILE, N], f32)
                nc.vector.tensor_scalar(rely[:], nei_v[:, :, 2 * T - 1], egoy, None, OP.subtract)

                sqx = geo.tile([TILE, N], f32)
                nc.scalar.square(sqx[:], relx[:])
                sqy = geo.tile([TILE, N], f32)
                nc.scalar.square(sqy[:], rely[:])
                d2 = geo.tile([TILE, N], f32)
                nc.vector.tensor_tensor(d2[:], sqx[:], sqy[:], op=OP.add)
                dist = geo.tile([TILE, N], f32)
                nc.scalar.sqrt(dist[:], d2[:])

                # atan2(relx, rely): y=relx, x=rely.
                # ACT Arctan domain is [-pi/2, pi/2] -> octant reduction:
                # at_r = atan(min(|x|,|y|)/max(|x|,|y|)) in [0, pi/4]
                ax = geo.tile([TILE, N], f32)   # |y| = |relx|
                nc.scalar.activation(ax[:], relx[:], AF.Abs)
                ay = geo.tile([TILE, N], f32)   # |x| = |rely|
                nc.scalar.activation(ay[:], rely[:], AF.Abs)
                mn = geo.tile([TILE, N], f32)
                nc.vector.tensor_tensor(mn[:], ax[:], ay[:], op=OP.min)
                mx = geo.tile([TILE, N], f32)
                nc.vector.tensor_tensor(mx[:], ax[:], ay[:], op=OP.max)
                scr = geo.tile([TILE, N], f32)
                invmx = geo.tile([TILE, N], f32)
                nc.vector.reciprocal_approx_accurate(out=invmx[:], in_=mx[:], scratch=scr[:])
                qr = geo.tile([TILE, N], f32)
                nc.vector.tensor_tensor(qr[:], mn[:], invmx[:], op=OP.mult)
                atr = geo.tile([TILE, N], f32)
                nc.scalar.activation(atr[:], qr[:], AF.Arctan)

                # le = (|y| <= |x|): atan(|q|) = le ? atr : pi/2 - atr
                le = geo.tile([TILE, N], f32)
                nc.vector.tensor_tensor(le[:], ax[:], ay[:], op=OP.is_le)
                u1 = geo.tile([TILE, N], f32)
                nc.vector.tensor_scalar(u1[:], atr[:], -1.0, float(np.float32(np.pi / 2)), OP.mult, OP.add)
                dd = geo.tile([TILE, N], f32)
                nc.vector.tensor_tensor(dd[:], atr[:], u1[:], op=OP.subtract)
                m1 = geo.tile([TILE, N], f32)
                nc.vector.tensor_tensor(m1[:], le[:], dd[:], op=OP.mult)
                aq = geo.tile([TILE, N], f32)   # atan(|q|) in [0, pi/2]
                nc.vector.tensor_tensor(aq[:], u1[:], m1[:], op=OP.add)

                # theta_abs = atan2(|y|, x) = xlt ? pi - aq : aq
                xlt = geo.tile([TILE, N], f32)
                nc.gpsimd.tensor_scalar(xlt[:], rely[:], 0.0, None, OP.is_lt)
                t2 = geo.tile([TILE, N], f32)
                nc.vector.tensor_scalar(t2[:], aq[:], -2.0, float(PI32), OP.mult, OP.add)
                m2 = geo.tile([TILE, N], f32)
                nc.vector.tensor_tensor(m2[:], xlt[:], t2[:], op=OP.mult)
                th = geo.tile([TILE, N], f32)
                nc.vector.tensor_tensor(th[:], aq[:], m2[:], op=OP.add)

                # dir = mod(atan2(y,x), 2pi) = ylt ? 2pi - theta_abs : theta_abs
                ylt = geo.tile([TILE, N], f32)
                nc.gpsimd.tensor_scalar(ylt[:], relx[:], 0.0, None, OP.is_lt)
                t3 = geo.tile([TILE, N], f32)
                nc.vector.tensor_scalar(t3[:], th[:], -2.0, float(TWOPI32), OP.mult, OP.add)
                m3 = geo.tile([TILE, N], f32)
                nc.vector.tensor_tensor(m3[:], ylt[:], t3[:], op=OP.mult)
                dirw = geo.tile([TILE, N], f32)
                nc.vector.tensor_tensor(dirw[:], th[:], m3[:], op=OP.add)

                # invalid neighbors (all-zero traj) -> dirm = -10 -> no bin
                eq0 = geo.tile([TILE, N], f32)
                nc.gpsimd.tensor_scalar(eq0[:], msum[:], 0.0, None, OP.is_equal)
                tmsk = geo.tile([TILE, N], f32)
                nc.vector.scalar_tensor_tensor(
                    out=tmsk[:], in0=dirw[:], scalar=10.0, in1=eq0[:],
                    op0=OP.add, op1=OP.mult,
                )
                dirm = geo.tile([TILE, N], f32)
                nc.vector.tensor_tensor(dirm[:], dirw[:], tmsk[:], op=OP.subtract)

                # --- binning ---
                ges = []
                for p in range(P + 1):
                    gep = geo.tile([TILE, N], f32, tag=f"ge{p}")
                    nc.vector.tensor_scalar(gep[:], dirm[:], float(THR[p]), None, OP.is_ge)
                    ges.append(gep)

                nvec = small.tile([TILE, P], f32)
                ohs = []
                for p in range(P):
                    ohp = geo.tile([TILE, N], f32, tag=f"oh{p}")
                    nc.vector.scalar_tensor_tensor(
                        out=ohp[:], in0=ges[p][:], scalar=0.0, in1=ges[p + 1][:],
                        op0=OP.add, op1=OP.subtract,
                        accum_out=nvec[:, p : p + 1],
                    )
                    ohs.append(ohp)

                nadj = small.tile([TILE, P], f32)
                nc.vector.tensor_scalar(nadj[:], nvec[:], 1e-4, None, OP.add)
                invn = small.tile([TILE, P], f32)
                nc.vector.reciprocal(invn[:], nadj[:])

                mdist = small.tile([TILE, P], f32)
                mdir = small.tile([TILE, P], f32)
                ohT = ohtp.tile([N, P * TILE], f32)
                tps = []
                for p in range(P):
                    ohsp = geo.tile([TILE, N], f32, tag=f"ohs{p}")
                    nc.vector.tensor_scalar(ohsp[:], ohs[p][:], invn[:, p : p + 1], None, OP.mult)

                    scr2 = geo.tile([TILE, N], f32, tag="scr2")
                    nc.vector.scalar_tensor_tensor(
                        out=scr2[:], in0=dist[:], scalar=0.0, in1=ohsp[:],
                        op0=OP.add, op1=OP.mult, accum_out=mdist[:, p : p + 1],
                    )
                    scr3 = geo.tile([TILE, N], f32, tag="scr3")
                    nc.vector.scalar_tensor_tensor(
                        out=scr3[:], in0=dirw[:], scalar=0.0, in1=ohsp[:],
                        op0=OP.add, op1=OP.mult, accum_out=mdir[:, p : p + 1],
                    )

                    if p % 4 == 0:
                        tp = tpsum.tile([128, 512], f32, tag=f"tp{p // 4}")
                        tps.append(tp)
                    nc.tensor.transpose(
                        tp[:, (p % 4) * TILE : (p % 4 + 1) * TILE], ohsp[:], ident[:]
                    )

                nc.scalar.copy(ohT[:, 0:512], tps[0][:])
                nc.scalar.copy(ohT[:, 512:1024], tps[1][:])

                # --- f_scan = relu(scan @ W + b), batched over samples ---
                fpre = geo.tile([TILE, P * D], f32, tag="fpre")
                for p in range(P):
                    t1 = geo.tile([TILE, D], f32, tag="t1")
                    nc.vector.scalar_tensor_tensor(
                        out=t1[:], in0=w0, scalar=mdist[:, p : p + 1], in1=bias,
                        op0=OP.mult, op1=OP.add,
                    )
                    nc.vector.scalar_tensor_tensor(
                        out=fpre[:, p * D : (p + 1) * D], in0=w1,
                        scalar=mdir[:, p : p + 1], in1=t1[:],
                        op0=OP.mult, op1=OP.add,
                    )
                fscan = geo.tile([TILE, P * D], f32, tag="fscan")
                nc.scalar.activation(fscan[:], fpre[:], AF.Relu)

                nc.sync.dma_start(out=fscan_out[rows, :], in_=fscan[:])

                # --- per-sample binning matmuls (flipped): out = f_res[b].T @ onehotS[b]
                # [64 d, 8 p] per sample; 2 samples on partition strips {0,64},
                # 64 samples along free -> whole 128-sample tile in ONE bank ---
                ohT_v = ohT[:].rearrange("n (p b) -> n p b", b=TILE)
                pres = opsum.tile([128, 512], f32, tag="pres")
                for s in range(TILE):
                    s2, s64 = s % 2, s // 2
                    nc.tensor.matmul(
                        pres[s2 * 64 : (s2 + 1) * 64, s64 * P : (s64 + 1) * P],
                        fres_sb[:, s * D : (s + 1) * D],
                        ohT_v[:, :, s],
                        start=True,
                        stop=True,
                    )
                stage = geo.tile([128, 512], f32, tag="stage")
                nc.scalar.copy(stage[:], pres[:])
                nc.sync.dma_start(
                    out=res_out[:, t * 512 : (t + 1) * 512], in_=stage[:]
                )

    nc.compile()
    return nc


def _get_program():
    if "nc" not in _prog_cache:
        _prog_cache["nc"] = _build_program()
    return _prog_cache["nc"]


def kernel(ego_traj_2d, nei_traj_2d, f_resonance, W_ce, b_ce):
    from concourse import bass_utils

    ego_traj_2d = np.asarray(ego_traj_2d, dtype=np.float32)
    nei_traj_2d = np.asarray(nei_traj_2d, dtype=np.float32)
    f_resonance = np.asarray(f_resonance, dtype=np.float32)
    W_ce = np.asarray(W_ce, dtype=np.float32)
    b_ce = np.asarray(b_ce, dtype=np.float32)

    nc = _get_program()

    wb_full = np.empty((TILE, 3 * D), dtype=np.float32)
    wb_full[:, 0:D] = W_ce[0]
    wb_full[:, D : 2 * D] = W_ce[1]
    wb_full[:, 2 * D : 3 * D] = b_ce

    ego_last = ego_traj_2d[:, -1, :]  # [B, 2]

    in_maps = []
    for c in range(NCORES):
        rows = slice(c * BC, (c + 1) * BC)
        nei_c = nei_traj_2d[rows].reshape(BC, N * T * 2)
        fresT_c = np.ascontiguousarray(
            f_resonance[rows].transpose(1, 0, 2)
        ).reshape(N, BC * D)
        egoR_c = np.ascontiguousarray(
            ego_last[rows].reshape(NT, TILE, 2).transpose(1, 0, 2)
        ).reshape(TILE, NT * 2)
        in_maps.append(
            {
                "nei": np.ascontiguousarray(nei_c),
                "fresT": fresT_c,
                "egoR": egoR_c,
                "wb": wb_full,
            }
        )

    res = bass_utils.run_bass_kernel_spmd(nc, in_maps, core_ids=list(range(NCORES)))
    outs = [
        decode_core(res.results[c]["res_out"], res.results[c]["fscan_out"])
        for c in range(NCORES)
    ]
    return np.concatenate(outs, axis=0)


def decode_core(res_raw, fscan_raw):
    """res_out row q = s2*64 + d, col = t*512 + s64*8 + p, sample b = t*128 + s64*2 + s2."""
    r = res_raw.reshape(2, D, NT, 64, P).transpose(2, 3, 0, 4, 1).reshape(BC, P, D)
    f = fscan_raw.reshape(BC, P, D)
    return np.concatenate([r, f], axis=-1)



# revision 14
# speedup vs baseline: 1.6082x; 1.6082x over previous
"""CircleLayer (histogram angle binning) Trainium2 Bass kernel, v2.

Full-input contract: kernel(**inputs) takes the complete arrays, shards the
batch dim across 8 NeuronCores (pure data parallel), runs one SPMD Bass
program, and gathers the full [B, P, 2*D] output.

v2 layout (512 samples/core = 4 tiles of 128, heavily rebalanced):
  - geometry on WIDE [128, 512] tiles (4 sample-tiles side by side on the
    free axis) to amortize DVE instruction overhead
  - validity-mask trajectory reduce split across Vector+GpSimd engines
  - per-(tile,bin) 1/n scaling moved to ACT (per-partition scale AP)
  - f_resonance shipped as bf16 (halves its HBM traffic; rel err ~4e-3,
    far under the 2e-2 gate); binning matmuls run bf16 with fp32 PSUM
  - binning matmuls process SAMPLE PAIRS: stationary = [fres_s|fres_s1]
    [128n, 128] bf16 (FWL-eligible), stream = [ohT interleaved (p,s2)]
    [128n, 16] -> halves PE instruction count; PSUM packs 32 pairs/bank
  - f_scan = relu(scan @ W + b) on the PE: scan rows gathered by SBUF->SBUF
    DMA reshape, stationary W [2,64], relu+bias folded into the ACT
    PSUM->SBUF copy (per-partition bias AP)
"""

import numpy as np

B, N, T, D = 4096, 128, 20, 64
P = 8
NCORES = 8
BC = B // NCORES  # samples per core
TILE = 128
NT = BC // TILE  # tiles per core
WF = NT * TILE  # 512, wide free size

PI32 = np.float32(np.pi)
TWOPI32 = np.float32(2.0 * np.pi)
C32 = np.float32((2.0 * np.pi) / P)

DO_MASK = True  # compute the all-zero-trajectory validity mask honestly


def _bin_thresholds():
    """T[p] = smallest fp32 x >= 0 with int32(fp32(x / C32)) >= p.

    Comparing dir >= T[p] then reproduces the reference's
    (dir / C32).astype(int32) binning exactly (fp32 division is monotone).
    """
    thr = [np.float32(0.0)]
    for p in range(1, P + 1):
        x = np.float32(np.float32(p) * C32)
        while int(np.float32(x / C32)) >= p:
            x = np.nextafter(x, np.float32(-np.inf))
        while int(np.float32(x / C32)) < p:
            x = np.nextafter(x, np.float32(np.inf))
        thr.append(np.float32(x))
    return thr


THR = _bin_thresholds()

_prog_cache = {}


def _build_program():
    import concourse.bass as bass
    import concourse.tile as tile
    from concourse import bacc, mybir
    from concourse.masks import make_identity

    f32 = mybir.dt.float32
    bf16 = mybir.dt.bfloat16
    AX = mybir.AxisListType
    OP = mybir.AluOpType
    AF = mybir.ActivationFunctionType

    nc = bacc.Bacc(
        "TRN2",
        target_bir_lowering=False,
        debug=False,
        enable_asserts=False,
        num_devices=NCORES,
    )

    FREE_NEI = N * T * 2  # 10240 floats per sample... per-neighbor 40

    if DO_MASK:
        nei = nc.dram_tensor("nei", [BC, FREE_NEI], f32, kind="ExternalInput").ap()
    neiLx = nc.dram_tensor("neiLx", [TILE, WF], f32, kind="ExternalInput").ap()
    neiLy = nc.dram_tensor("neiLy", [TILE, WF], f32, kind="ExternalInput").ap()
    fresT = nc.dram_tensor("fresT", [N, BC * D], bf16, kind="ExternalInput").ap()
    egoR = nc.dram_tensor("egoR", [TILE, NT * 2], f32, kind="ExternalInput").ap()
    wst = nc.dram_tensor("wst", [2, D], bf16, kind="ExternalInput").ap()
    biasP = nc.dram_tensor("biasP", [TILE, 1], f32, kind="ExternalInput").ap()
    res_out = nc.dram_tensor("res_out", [128, NT * 512], f32, kind="ExternalOutput").ap()
    fscT_out = nc.dram_tensor("fscT_out", [128, NT * 512], f32, kind="ExternalOutput").ap()

    with tile.TileContext(nc) as tc:
        with (
            tc.tile_pool(name="const", bufs=1) as constp,
            tc.tile_pool(name="neip", bufs=2) as neip,
            tc.tile_pool(name="fresp", bufs=2) as fresp,
            tc.tile_pool(name="geo", bufs=1) as geo,
            tc.tile_pool(name="oht", bufs=2) as ohtp,
            tc.tile_pool(name="outp", bufs=2) as outp,
            tc.tile_pool(name="tpsum", bufs=1, space="PSUM") as tpsum,
            tc.tile_pool(name="opsum", bufs=2, space="PSUM") as opsum,
            tc.tile_pool(name="fpsum", bufs=2, space="PSUM") as fpsum,
        ):
            ident = constp.tile([128, 128], f32)
            make_identity(nc, ident[:])
            ego_sb = constp.tile([TILE, NT * 2], f32)
            nc.sync.dma_start(out=ego_sb[:], in_=egoR)
            wst_sb = constp.tile([2, D], bf16)
            nc.sync.dma_start(out=wst_sb[:], in_=wst)
            biasP_sb = constp.tile([TILE, 1], f32)
            nc.sync.dma_start(out=biasP_sb[:], in_=biasP)
            nlx = constp.tile([TILE, WF], f32)
            nc.sync.dma_start(out=nlx[:], in_=neiLx)
            nly = constp.tile([TILE, WF], f32)
            nc.sync.dma_start(out=nly[:], in_=neiLy)

            def wt(tag):
                # 12 reusable [128, 512] slots; callers pass the slot name.
                return geo.tile([TILE, WF], f32, tag=tag, name=tag)

            TS = slice  # alias

            def tcols(t):
                return slice(t * TILE, (t + 1) * TILE)

            # ---- mask sums (per tile) ----
            if DO_MASK:
                msum = wt("W0")
                nei_sbs = []
                for t in range(NT):
                    nei_sb = neip.tile([TILE, FREE_NEI], f32, tag=f"nei{t % 2}")
                    nc.sync.dma_start(out=nei_sb[:], in_=nei[tcols(t), :])
                    nei_v = nei_sb[:].rearrange("b (n f) -> b n f", f=T * 2)
                    nc.vector.tensor_reduce(msum[:, tcols(t)], nei_v, axis=AX.X, op=OP.add)

            # ---- geometry (wide) ----
            relx = wt("W1")
            rely = wt("W2")
            for t in range(NT):
                nc.vector.tensor_scalar(
                    relx[:, tcols(t)], nlx[:, tcols(t)],
                    ego_sb[:, 2 * t : 2 * t + 1], None, OP.subtract,
                )
                nc.vector.tensor_scalar(
                    rely[:, tcols(t)], nly[:, tcols(t)],
                    ego_sb[:, 2 * t + 1 : 2 * t + 2], None, OP.subtract,
                )

            sqx = wt("W3")
            nc.vector.tensor_tensor(sqx[:], relx[:], relx[:], op=OP.mult)
            sqy = wt("W4")
            nc.vector.tensor_tensor(sqy[:], rely[:], rely[:], op=OP.mult)
            d2 = wt("W5")
            nc.vector.tensor_tensor(d2[:], sqx[:], sqy[:], op=OP.add)
            dist = wt("W6")
            nc.scalar.activation(dist[:], d2[:], AF.Sqrt)

            ax = wt("W3")
            nc.scalar.activation(ax[:], relx[:], AF.Abs)
            ay = wt("W4")
            nc.scalar.activation(ay[:], rely[:], AF.Abs)
            mn = wt("W7")
            nc.vector.tensor_tensor(mn[:], ax[:], ay[:], op=OP.min)
            mx = wt("W8")
            nc.vector.tensor_tensor(mx[:], ax[:], ay[:], op=OP.max)
            scr = wt("W9")
            invmx = wt("W10")
            nc.vector.reciprocal_approx_accurate(out=invmx[:], in_=mx[:], scratch=scr[:])
            qr = wt("W11")
            nc.vector.tensor_tensor(qr[:], mn[:], invmx[:], op=OP.mult)
            # le before d2's slot is reused is not needed; le gets its own slot
            le = wt("W5")
            nc.vector.tensor_tensor(le[:], ax[:], ay[:], op=OP.is_le)
            atr = wt("W7")
            nc.scalar.activation(atr[:], qr[:], AF.Arctan)

            # aq = atan(|y|/|x| octant-corrected) = u1 + le*dd
            #   u1 = pi/2 - atr ; dd = 2*atr - pi/2 ; le = (|y| <= |x|)
            u1 = wt("W8")
            nc.scalar.activation(u1[:], atr[:], AF.Copy, scale=-1.0,
                                 bias=float(np.float32(np.pi / 2)))
            dd = wt("W9")
            nc.scalar.activation(dd[:], atr[:], AF.Copy, scale=2.0,
                                 bias=float(np.float32(-np.pi / 2)))
            m1 = wt("W10")
            nc.vector.tensor_tensor(m1[:], le[:], dd[:], op=OP.mult)
            aq = wt("W11")
            nc.vector.tensor_tensor(aq[:], u1[:], m1[:], op=OP.add)

            # th = xlt ? pi - aq : aq   (xlt = x<0, x=rely)
            xlt = wt("W5")
            nc.vector.tensor_scalar(xlt[:], rely[:], 0.0, None, OP.is_lt)
            t2 = wt("W7")
            nc.scalar.activation(t2[:], aq[:], AF.Copy, scale=-2.0, bias=float(PI32))
            m2 = wt("W8")
            nc.vector.tensor_tensor(m2[:], xlt[:], t2[:], op=OP.mult)
            th = wt("W9")
            nc.vector.tensor_tensor(th[:], aq[:], m2[:], op=OP.add)

            # dir = ylt ? 2pi - th : th   (ylt = y<0, y=relx)
            ylt = wt("W5")
            nc.vector.tensor_scalar(ylt[:], relx[:], 0.0, None, OP.is_lt)
            t3 = wt("W7")
            nc.scalar.activation(t3[:], th[:], AF.Copy, scale=-2.0, bias=float(TWOPI32))
            m3 = wt("W8")
            nc.vector.tensor_tensor(m3[:], ylt[:], t3[:], op=OP.mult)
            dirw = wt("W10")
            nc.vector.tensor_tensor(dirw[:], th[:], m3[:], op=OP.add)

            # invalid neighbors (all-zero traj) -> dirm = -10 -> no bin
            if DO_MASK:
                dirm = wt("W11")
                for t in range(NT):
                    c = tcols(t)
                    eq0 = geo.tile([TILE, TILE], f32, tag="eq0")
                    nc.vector.tensor_scalar(eq0[:], msum[:, c], 0.0, None, OP.is_equal)
                    tmsk = geo.tile([TILE, TILE], f32, tag="tmsk")
                    nc.vector.scalar_tensor_tensor(
                        out=tmsk[:], in0=dirw[:, c], scalar=10.0, in1=eq0[:],
                        op0=OP.add, op1=OP.mult,
                    )
                    nc.vector.tensor_tensor(dirm[:, c], dirw[:, c], tmsk[:], op=OP.subtract)
            else:
                dirm = dirw

            # ---- binning: ge thresholds with per-tile counts ----
            GE_SLOTS = ["W0", "W1", "W2", "W3", "W4", "W5", "W7", "W8", "W9"]
            ges = [wt(GE_SLOTS[p]) for p in range(P + 1)]
            Gt = geo.tile([TILE, NT * 9], f32, tag="Gt")
            for t in range(NT):
                for p in range(P + 1):
                    nc.vector.tensor_scalar(
                        ges[p][:, tcols(t)], dirm[:, tcols(t)], float(THR[p]), None,
                        OP.is_ge, OP.add,
                        accum_out=Gt[:, t * 9 + p : t * 9 + p + 1],
                    )

            # n per (tile, bin) and 1/(n+1e-4), [128, NT*8]
            Gv = Gt[:].rearrange("b (t q) -> b t q", q=9)
            nv = geo.tile([TILE, NT * 8], f32, tag="nv")
            nv_v = nv[:].rearrange("b (t p) -> b t p", p=8)
            nc.vector.tensor_tensor(nv_v, Gv[:, :, 0:8], Gv[:, :, 1:9], op=OP.subtract)
            nadj = geo.tile([TILE, NT * 8], f32, tag="nadj")
            nc.vector.tensor_scalar(nadj[:], nv[:], 1e-4, None, OP.add)
            invn = geo.tile([TILE, NT * 8], f32, tag="invn")
            nc.vector.reciprocal(invn[:], nadj[:])

            # ohn_p = ge_{p+1} - ge_p = -onehot_p  (wide); ohn_p reuses the
            # slot of ge_{p-1} (dead once ohn_{p-1} is computed); ohn_0 takes
            # dirm's slot (dead after the ge compares).
            ohns = []
            for p in range(P):
                ohn = wt("W11" if p == 0 else GE_SLOTS[p - 1])
                nc.vector.tensor_tensor(ohn[:], ges[p + 1][:], ges[p][:], op=OP.subtract)
                ohns.append(ohn)

            # negated sums: mdraw = -sum_dist per (tile,bin); mdirw = -sum_dir
            mdraw = geo.tile([TILE, NT * 8], f32, tag="mdraw")
            mdirw = geo.tile([TILE, NT * 8], f32, tag="mdirw")
            scrap = geo.tile([TILE, TILE], f32, tag="scrap")
            scrap2 = geo.tile([TILE, TILE], f32, tag="scrap2")
            for t in range(NT):
                for p in range(P):
                    i = t * 8 + p
                    nc.vector.scalar_tensor_tensor(
                        out=scrap[:], in0=dist[:, tcols(t)], scalar=0.0,
                        in1=ohns[p][:, tcols(t)], op0=OP.add, op1=OP.mult,
                        accum_out=mdraw[:, i : i + 1],
                    )
                    nc.vector.scalar_tensor_tensor(
                        out=scrap2[:], in0=dirw[:, tcols(t)], scalar=0.0,
                        in1=ohns[p][:, tcols(t)], op0=OP.add, op1=OP.mult,
                        accum_out=mdirw[:, i : i + 1],
                    )

            # negated means (still negative; cancels with wst = -W)
            mdn = geo.tile([TILE, NT * 8], f32, tag="mdn")
            nc.vector.tensor_tensor(mdn[:], mdraw[:], invn[:], op=OP.mult)
            mdirn = geo.tile([TILE, NT * 8], f32, tag="mdirn")
            nc.vector.tensor_tensor(mdirn[:], mdirw[:], invn[:], op=OP.mult)
            mdn_bf = geo.tile([TILE, NT * 8], bf16, tag="mdn_bf")
            nc.vector.tensor_scalar(mdn_bf[:], mdn[:], 0.0, None, OP.add)
            mdir_bf = geo.tile([TILE, NT * 8], bf16, tag="mdir_bf")
            nc.vector.tensor_scalar(mdir_bf[:], mdirn[:], 0.0, None, OP.add)

            # scan stream rows via SBUF->SBUF DMA reshape: [2, NT*1024] bf16
            scan_sb = geo.tile([2, NT * TILE * 8], bf16, tag="scan")
            mdn_v = mdn_bf[:].rearrange("b (t p) -> b t p", p=8)
            mdir_v = mdir_bf[:].rearrange("b (t p) -> b t p", p=8)
            for t in range(NT):
                o = t * TILE * 8
                nc.sync.dma_start(out=scan_sb[0:1, o : o + TILE * 8], in_=mdn_v[:, t, :])
                nc.sync.dma_start(out=scan_sb[1:2, o : o + TILE * 8], in_=mdir_v[:, t, :])

            # f_scan = relu((-W)@(-scan) + b) : PE matmul + ACT relu w/ bias AP
            for t in range(NT):
                fsp = fpsum.tile([128, 512], f32, tag="fsp")
                o = t * TILE * 8
                nc.tensor.matmul(
                    fsp[0:64, :], wst_sb[:], scan_sb[:, o : o + 512],
                    start=True, stop=True,
                )
                nc.tensor.matmul(
                    fsp[64:128, :], wst_sb[:], scan_sb[:, o + 512 : o + 1024],
                    start=True, stop=True,
                )
                fsc = outp.tile([128, 512], f32, tag="fsc")
                nc.scalar.activation(fsc[:], fsp[:], AF.Relu, bias=biasP_sb[:, 0:1])
                nc.sync.dma_start(out=fscT_out[:, t * 512 : (t + 1) * 512], in_=fsc[:])

            # ---- per-tile: scale oh, transpose, pair matmuls, compact ----
            for t in range(NT):
                fres_sb = fresp.tile([N, TILE * D], bf16, tag=f"fres{t % 2}")
                nc.sync.dma_start(
                    out=fres_sb[:], in_=fresT[:, t * TILE * D : (t + 1) * TILE * D]
                )

                # ohsp_p = ohn_p * (1/n) on ACT (per-partition scale AP)
                tps = []
                for p in range(P):
                    ohsp = geo.tile([TILE, TILE], f32, tag=f"ohsp{p}")
                    nc.scalar.activation(
                        ohsp[:], ohns[p][:, tcols(t)], AF.Copy,
                        scale=invn[:, t * 8 + p : t * 8 + p + 1],
                    )
                    if p % 4 == 0:
                        tp = tpsum.tile([128, 512], f32, tag=f"tp{p // 4}")
                        tps.append(tp)
                    nc.tensor.transpose(
                        tp[:, (p % 4) * TILE : (p % 4 + 1) * TILE], ohsp[:], ident[:]
                    )

                ohT = ohtp.tile([N, P * TILE], bf16, tag="ohT")
                nc.scalar.copy(out=ohT[:, 0:512], in_=tps[0][:])
                nc.scalar.copy(out=ohT[:, 512:1024], in_=tps[1][:])
                ohT_v = ohT[:].rearrange("n (p s) -> n p s", s=TILE)

                # pair matmuls: stationary [fres_s|fres_s1] [128,128] bf16 (FWL)
                banks = [
                    opsum.tile([128, 512], f32, tag="presA", name="presA"),
                    opsum.tile([128, 512], f32, tag="presB", name="presB"),
                ]
                for pr in range(TILE // 2):
                    bank = banks[pr // 32]
                    col = (pr % 32) * 16
                    nc.tensor.matmul(
                        bank[:, col : col + 16],
                        fres_sb[:, (2 * pr) * D : (2 * pr) * D + 128],
                        ohT_v[:, :, 2 * pr : 2 * pr + 2],
                        start=True, stop=True,
                    )

                # compact diag blocks -> stage [128, 512]; negate (ohsp = -oh/n)
                stage = outp.tile([128, 512], f32, tag="stage")
                st_v = stage[:].rearrange("q (b pr p) -> q b pr p", b=2, p=8)
                for b in range(2):
                    bv = banks[b][:].rearrange("q (pr p s) -> q pr p s", p=8, s=2)
                    nc.scalar.activation(
                        st_v[0:64, b, :, :], bv[0:64, :, :, 0], AF.Copy, scale=-1.0
                    )
                    nc.scalar.activation(
                        st_v[64:128, b, :, :], bv[64:128, :, :, 1], AF.Copy, scale=-1.0
                    )
                nc.sync.dma_start(out=res_out[:, t * 512 : (t + 1) * 512], in_=stage[:])

    nc.compile()
    return nc


def _get_program():
    if "nc" not in _prog_cache:
        _prog_cache["nc"] = _build_program()
    return _prog_cache["nc"]


def _make_in_maps(ego_traj_2d, nei_traj_2d, f_resonance, W_ce, b_ce):
    import ml_dtypes

    bf16 = ml_dtypes.bfloat16
    ego_last = ego_traj_2d[:, -1, :]  # [B, 2]
    nei_last = nei_traj_2d[:, :, -1, :]  # [B, N, 2]

    wst = np.ascontiguousarray(-W_ce).astype(bf16)  # [2, D]
    biasP = np.concatenate([b_ce, b_ce]).reshape(TILE, 1).astype(np.float32)

    in_maps = []
    for c in range(NCORES):
        rows = slice(c * BC, (c + 1) * BC)
        nl = nei_last[rows].reshape(NT, TILE, N, 2).transpose(1, 0, 2, 3)
        # [128, NT, N, 2]
        neiLx = np.ascontiguousarray(nl[..., 0]).reshape(TILE, WF)
        neiLy = np.ascontiguousarray(nl[..., 1]).reshape(TILE, WF)
        fresT_c = np.ascontiguousarray(
            f_resonance[rows].transpose(1, 0, 2)
        ).reshape(N, BC * D).astype(bf16)
        egoR_c = np.ascontiguousarray(
            ego_last[rows].reshape(NT, TILE, 2).transpose(1, 0, 2)
        ).reshape(TILE, NT * 2)
        m = {
            "neiLx": neiLx,
            "neiLy": neiLy,
            "fresT": fresT_c,
            "egoR": egoR_c,
            "wst": wst,
            "biasP": biasP,
        }
        if DO_MASK:
            m["nei"] = np.ascontiguousarray(
                nei_traj_2d[rows].reshape(BC, N * T * 2)
            )
        in_maps.append(m)
    return in_maps


def kernel(ego_traj_2d, nei_traj_2d, f_resonance, W_ce, b_ce):
    from concourse import bass_utils

    ego_traj_2d = np.asarray(ego_traj_2d, dtype=np.float32)
    nei_traj_2d = np.asarray(nei_traj_2d, dtype=np.float32)
    f_resonance = np.asarray(f_resonance, dtype=np.float32)
    W_ce = np.asarray(W_ce, dtype=np.float32)
    b_ce = np.asarray(b_ce, dtype=np.float32)

    nc = _get_program()
    in_maps = _make_in_maps(ego_traj_2d, nei_traj_2d, f_resonance, W_ce, b_ce)
    res = bass_utils.run_bass_kernel_spmd(nc, in_maps, core_ids=list(range(NCORES)))
    outs = [
        decode_core(res.results[c]["res_out"], res.results[c]["fscT_out"])
        for c in range(NCORES)
    ]
    return np.concatenate(outs, axis=0)


def decode_core(res_raw, fscT_raw):
    """res_out row q = s2*64 + d, col = t*512 + pr*8 + p, sample = t*128 + 2*pr + s2.
    fscT row r = hi*64 + d, col = t*512 + sl*8 + p, sample = t*128 + hi*64 + sl."""
    r = res_raw.reshape(2, D, NT, 64, P).transpose(2, 3, 0, 4, 1).reshape(BC, P, D)
    f = fscT_raw.reshape(2, D, NT, 64, P).transpose(2, 0, 3, 4, 1).reshape(BC, P, D)
    return np.concatenate([r, f], axis=-1)
